# revision 5
# baseline (speedup 1.0000x reference)
"""Multi-head attention (B=1, n=4096, d=768, H=12) on 8 Trainium2 NeuronCores.

Sharding: 4 head-groups (3 heads = 192 dims) x 2 query-halves (2048 q).
Core c = (hg = c // 2, sq = c % 2).

Cost-model-driven design (TimelineSim charges a matmul = moving-row count
regardless of stationary width, so output partitions should be 128 wide):

Per core:
  kT = Wk_hg @ K^T          [192, 4096]   2 partition-chunks (128+64)
  qT = Wq_hg @ Q_sq^T       [192, 2048]
  v  = V @ Wv_hg^T (+ ones) [4096, 3*65]  (65th col = 1.0 per head)
  Per phase (head h, 256-query chunk qc), 24 phases:
    S^T[kpos128, q256] = kT_h^T q_h       (dk=64 contraction, 8 PSUM tiles
                                           of 4 key-tiles each)
    e = exp(S^T/8)    ACT engine, bf16; 3 of 8 tiles per phase (j=2,5,7)
                      computed instead on DVE via a Schraudolph bf16-bits
                      fast exp (i16 round-convert of A*s+B, bitcast to bf16;
                      RMS rel err 1.8% on 37.5% of keys -> ~1.1% output
                      contribution, inside the 2e-2 gate)
    attnV (next phase, E-stationary): acc[q128, 65] += e[:,qb]^T [v_h|1]
      as 2 sequential 32-step PSUM chains (65-row matmuls, full 128 output
      partitions - half the PE cost of the v-stationary form)
    normalize: r = 1/acc[:,64] (DVE), o = acc[:,:64]*r -> bf16,
      PE-transpose (identity) -> oT[dims, q], DVE copy to oT_sb
  Y_part[2048, 768] = oT^T @ WoT_hg  (two dim-chunks accumulated in PSUM)

Host: Y[sq-half] = sum over 4 head-groups + bo.

Projections/out-proj are spread into the attention phases as PE filler so
the ACT exp stream and PE stay co-busy; PSUM: 3x2-bank tiles for scores/
proj/out-proj + 2x1-bank phase tiles (attnV accumulators + transpose area).
"""

import os as _os
import numpy as np
import ml_dtypes
KDEBUG = bool(_os.environ.get("KDEBUG"))

import concourse.bass as bass  # noqa: F401
import concourse.mybir as mybir
import concourse.tile as tile
from concourse import bacc
from concourse.bass_utils import run_bass_kernel_spmd

P = 128
D = 768
NPOS = 4096
NQ = 2048          # queries per core
KD = D // P        # 6 contraction tiles for projections
NKT = NPOS // P    # 32 key-position tiles
DK = 64
VW = 65            # v columns per head incl. ones column
QCH = 256          # query chunk per attention phase
NJ = 8             # j-steps per phase (4 key-tiles each)
HC = 3             # heads per core
NPH = HC * NPOS // QCH // 2  # 24 phases (qc covers NQ=2048 -> 8 chunks/head)
BF16 = mybir.dt.bfloat16
F32 = mybir.dt.float32
I16 = mybir.dt.int16
FP = mybir.ActivationFunctionType
ALU = mybir.AluOpType

# head h -> (dim-chunk, partition offset)
HSL = [(0, 0), (0, 64), (1, 0)]
CW = [128, 64]     # chunk widths

# Schraudolph fast-exp constants (bf16 bits = round(s*EXPA + EXPB)), folding
# the 1/sqrt(dk)=1/8 score scale; calibrated for round-to-nearest convert.
EXPA = float(16.0 * np.log2(np.e))
EXPB = float(127.0 * 128.0 - 7.43)
FAST_EVERY = 4     # every FAST_EVERY-th exp tile goes to DVE fast-exp

_CACHED_NC = None
LAST_RESULTS = None  # BassKernelResults from the most recent run (for test.py)


def build_program():
    nc = bacc.Bacc("TRN2", target_bir_lowering=False, debug=False)

    KT = nc.dram_tensor("KT", [D, NPOS], BF16, kind="ExternalInput")
    VTb4 = nc.dram_tensor("VTb4", [NKT // 4, P, 4 * D], BF16, kind="ExternalInput")
    QT = nc.dram_tensor("QT", [D, NQ], BF16, kind="ExternalInput")
    # weights pre-packed on host to [128, k*m] (partition-major) for
    # contiguous DMA runs
    NB2 = 2 * (4 + HC * DK)   # bias f32 block carried as bf16 columns
    WKB = nc.dram_tensor("WKB", [P, KD * HC * DK + NB2], BF16, kind="ExternalInput")
    WQV = nc.dram_tensor("WQV", [P, 2 * KD * HC * DK], BF16, kind="ExternalInput")
    WoT = nc.dram_tensor("WoT", [P, 2 * D], BF16, kind="ExternalInput")
    IDN = nc.dram_tensor("IDN", [P, P], BF16, kind="ExternalInput")
    Y = nc.dram_tensor("Y", [NQ, D], F32, kind="ExternalOutput")
    if KDEBUG:
        KTD = nc.dram_tensor("KTD", [P, 2, NPOS], BF16, kind="ExternalOutput")
        QTD = nc.dram_tensor("QTD", [P, 2, NQ], BF16, kind="ExternalOutput")
        VD = nc.dram_tensor("VD", [P, NKT, HC * VW], BF16, kind="ExternalOutput")
        OTD = nc.dram_tensor("OTD", [P, 2, NQ], BF16, kind="ExternalOutput")
        VQD = nc.dram_tensor("VQD", [P, 4 * D], BF16, kind="ExternalOutput")

    with tile.TileContext(nc) as tc:
        with (
            tc.tile_pool(name="const", bufs=1) as const,
            tc.tile_pool(name="persist", bufs=1) as persist,
            tc.tile_pool(name="vin", bufs=4) as vin,
            tc.tile_pool(name="epool", bufs=17) as epool,
            tc.tile_pool(name="small", bufs=4) as small,
            tc.tile_pool(name="ypool", bufs=2) as ypool,
            tc.tile_pool(name="ps_s", bufs=3, space="PSUM") as ps_s,
            tc.tile_pool(name="ps_ph", bufs=2, space="PSUM") as ps_ph,
        ):
            # ---- SBUF tiles (DMAs emitted in need-order below) ----
            wqv_sb = const.tile([P, 2, KD, HC * DK], BF16)
            wq_sb = wqv_sb[:, 0]
            wv_sb = wqv_sb[:, 1]
            wkb_sb = const.tile([P, KD * HC * DK + NB2], BF16)
            wk_sb = wkb_sb[:, 0:KD * HC * DK].rearrange(
                "p (k m) -> p k m", k=KD)
            bias_sb = wkb_sb[:, KD * HC * DK:].bitcast(F32)
            bk_sb = bias_sb[:, 0:2]
            bq_sb = bias_sb[:, 2:4]
            bvr_sb = bias_sb[:, 4:]
            wo_sb = const.tile([P, 2, D], BF16)
            idn = const.tile([P, P], BF16)

            KT_res = persist.tile([P, KD, NPOS], BF16)
            QT_res = persist.tile([P, KD, NQ], BF16)
            kT_sb = persist.tile([P, 2, NPOS], BF16)
            qT_sb = persist.tile([P, 2, NQ], BF16)
            v_sb = persist.tile([P, NKT, HC * VW], BF16)
            oT_sb = persist.tile([P, 2, NQ], BF16)

            v_heads = v_sb.rearrange("p m (h x) -> p m h x", x=VW)
            nc.vector.memset(v_heads[:, :, :, DK], 1.0)

            KT_r = KT.rearrange("(k p) (t n) -> p k t n", p=P, n=512)
            QT_r = QT.rearrange("(k p) (t n) -> p k t n", p=P, n=512)

            # SP queue: wk, biases, KT pieces, wq/wv; Pool-issued (SWDGE)
            # queue: idn, QT pieces, VTb quads, wo. Host pre-packs weights
            # partition-major so every DMA has long contiguous runs.
            nc.sync.dma_start(wkb_sb, WKB[:, :])
            nc.sync.dma_start(KT_res[:, :, 0:512], KT_r[:, :, 0])
            nc.sync.dma_start(
                wqv_sb.rearrange("p a k m -> p (a k m)"), WQV[:, :])
            nc.gpsimd.dma_start(QT_res[:, :, 0:512], QT_r[:, :, 0])
            nc.gpsimd.dma_start(idn, IDN[:, :])

            def load_kt(nt):
                nc.sync.dma_start(KT_res[:, :, nt * 512:(nt + 1) * 512], KT_r[:, :, nt])

            def load_kt2(pc):  # 1024-wide KT piece (nt 2pc+1, 2pc+2)
                nc.sync.dma_start(
                    KT_res[:, :, (2 * pc + 1) * 512:(2 * pc + 3) * 512],
                    KT_r.rearrange("p k t n -> p k (t n)")[
                        :, :, (2 * pc + 1) * 512:(2 * pc + 3) * 512],
                )

            def load_qt(nt):
                nc.gpsimd.dma_start(QT_res[:, :, nt * 512:(nt + 1) * 512], QT_r[:, :, nt])

            vts = {}

            def v_load4(qd):  # quad of VTb tiles on the Pool/SWDGE queue
                vtq = vin.tile([P, 4, D], BF16, tag="vin")
                nc.gpsimd.dma_start(vtq.rearrange("p m d -> p (m d)"), VTb4[qd])
                for i in range(4):
                    vts[4 * qd + i] = vtq[:, i, :]

            # trigger the exp table load while input DMAs stream
            warm_in = const.tile([1, DK], F32)
            nc.vector.memset(warm_in, 0.0)
            warm_sb = const.tile([1, DK], F32)
            nc.scalar.activation(warm_sb, warm_in, FP.Exp)

            def proj_chunk(c, w_sb, b_sb, dst, nt):
                """Project 512 positions of dim-chunk c of k or q (+bias)."""
                src = KT_res if dst is kT_sb else QT_res
                cw = CW[c]
                ps = ps_s.tile([P, 512], F32, tag="s2")
                for kt in range(KD):
                    nc.tensor.matmul(
                        ps[0:cw, :], w_sb[:, kt, c * P:c * P + cw],
                        src[:, kt, nt * 512:(nt + 1) * 512],
                        start=(kt == 0), stop=(kt == KD - 1),
                    )
                nc.vector.tensor_scalar_add(
                    dst[0:cw, c, nt * 512:(nt + 1) * 512], ps[0:cw, :],
                    b_sb[0:cw, c:c + 1],
                )

            def v_proj(mt):
                vt = vts.pop(mt)  # [P, D] slice of a quad tile
                ps = ps_s.tile([P, 512], F32, tag="s2")
                for kt in range(KD):
                    nc.tensor.matmul(
                        ps[:, 0:HC * DK], vt[:, kt * P:(kt + 1) * P],
                        wv_sb[:, kt, :],
                        start=(kt == 0), stop=(kt == KD - 1),
                    )
                nc.vector.tensor_tensor(
                    v_heads[:, mt, :, 0:DK],
                    ps[:, 0:HC * DK].rearrange("p (h x) -> p h x", x=DK),
                    bvr_sb.rearrange("p (h x) -> p h x", x=DK),
                    ALU.add,
                )

            ypend = {}

            def out_proj_blk(qt, blk):
                if blk == 0:
                    ypend[qt] = ypool.tile([P, D], F32, tag="y", name="y_sb")
                y_sb = ypend[qt]
                n0, nsz = ((0, 512), (512, 256))[blk]
                ps = ps_s.tile([P, 512], F32, tag="s2")
                nc.tensor.matmul(
                    ps[:, 0:nsz], oT_sb[:, 0, qt * P:(qt + 1) * P],
                    wo_sb[:, 0, n0:n0 + nsz], start=True, stop=False,
                )
                nc.tensor.matmul(
                    ps[:, 0:nsz], oT_sb[0:DK, 1, qt * P:(qt + 1) * P],
                    wo_sb[0:DK, 1, n0:n0 + nsz], start=False, stop=True,
                )
                nc.vector.tensor_copy(y_sb[:, n0:n0 + nsz], ps[:, 0:nsz])
                if blk == 1:
                    nc.sync.dma_start(Y[qt * P:(qt + 1) * P, :], ypend.pop(qt))

            def out_proj(qt):
                out_proj_blk(qt, 0)
                out_proj_blk(qt, 1)

            # ---- phase machinery ----
            class Ph:
                def __init__(self, h, qc, etiles):
                    self.h, self.qc, self.e = h, qc, etiles
                    self.pht = None

            def emit_chains_j(prev, j):
                """8 attnV matmuls of the previous phase at j-step j."""
                h, qc = prev.h, prev.qc
                qb = j // 4
                for m in range(8):
                    kt = 8 * (j % 4) + m
                    et = prev.e[kt // 4]
                    nc.tensor.matmul(
                        prev.pht[:, qb * VW:qb * VW + VW],
                        et[:, kt % 4, qb * P:(qb + 1) * P],
                        v_heads[:, kt, h, :],
                        start=(kt == 0), stop=(kt == NKT - 1),
                    )

            def normalize_a(prev):
                """DVE part: reciprocal of the ones-column, scale to bf16."""
                prev.on = []
                for qb in range(2):
                    r = small.tile([P, 1], F32, tag="r")
                    with nc.allow_low_precision(reason="softmax reciprocal"):
                        nc.vector.reciprocal(
                            r, prev.pht[:, qb * VW + DK:qb * VW + DK + 1])
                    o_n = small.tile([P, DK], BF16, tag="on")
                    nc.vector.tensor_scalar(
                        o_n, prev.pht[:, qb * VW:qb * VW + DK], r, None, ALU.mult)
                    prev.on.append(o_n)

            def normalize_b(prev):
                """PE transpose (deferred so PE never waits on the DVE part)
                + DVE copy into oT_sb."""
                h, qc = prev.h, prev.qc
                c, poff = HSL[h]
                for qb in range(2):
                    tp = prev.pht[poff:poff + DK,
                                  384 + qb * DK:384 + (qb + 1) * DK].bitcast(BF16)
                    nc.tensor.transpose(tp, prev.on[qb], idn)
                    nc.vector.tensor_copy(
                        oT_sb[poff:poff + DK, c,
                              qc * QCH + qb * P:qc * QCH + (qb + 1) * P],
                        tp,
                    )

            # ---- filler schedule: (phase, j) -> list of callbacks ----
            fill = {}

            def add_fill(p, j, cb):
                fill.setdefault((p, j), []).append(cb)

            def kchunk(c, nt):
                return lambda: proj_chunk(c, wk_sb, bk_sb, kT_sb, nt)

            def qchunk(c, nt):
                return lambda: proj_chunk(c, wq_sb, bq_sb, qT_sb, nt)

            # phase 0: kT chunk0 JIT (KT arriving in 1024-wide pieces) plus
            # the first 14 v projections; VTb quads staged on the Pool queue.
            for pc in range(3):
                add_fill(0, 2 * pc, lambda n=pc: load_kt2(n))
            add_fill(0, 0, lambda: load_kt(7))
            for j in range(8):
                add_fill(0, j, kchunk(0, j))
            add_fill(0, 0, qchunk(0, 0))
            add_fill(0, 1, lambda: v_load4(2))
            add_fill(0, 3, lambda: v_load4(3))
            add_fill(0, 5, lambda: v_load4(4))
            add_fill(0, 7, lambda: v_load4(5))
            for j in range(1, 8):
                add_fill(0, j, lambda m=2 * j - 2: v_proj(m))
                add_fill(0, j, lambda m=2 * j - 1: v_proj(m))
            # phase 1: v14..31 (5,5,5,3 over j0..3 - chain j needs v<8(j+1))
            add_fill(1, 0, lambda: load_qt(1))
            add_fill(1, 0, lambda: v_load4(6))
            add_fill(1, 1, lambda: v_load4(7))
            vnext = 14
            for j in range(4):
                for _ in range(5 if j < 3 else 3):
                    add_fill(1, j, lambda m=vnext: v_proj(m))
                    vnext += 1
            add_fill(1, 5, qchunk(0, 1))      # needed by phase 2
            add_fill(2, 0, lambda: load_qt(2))
            add_fill(3, 5, qchunk(0, 2))
            add_fill(3, 0, lambda: load_qt(3))
            add_fill(5, 5, qchunk(0, 3))
            for i in range(8):                # kT chunk1: needed by phase 16
                add_fill(2 + i, 5, kchunk(1, i))
            for i in range(4):                # qT chunk1: needed by phase 16+2i
                add_fill(10 + i, 5, qchunk(1, i))
            add_fill(15, 5, lambda: nc.gpsimd.dma_start(
                wo_sb.rearrange("p k m -> p (k m)"), WoT[:, :]))
            for qc in range(6):               # out-proj qt pairs, phases 18..23
                add_fill(18 + qc, 2, lambda q=2 * qc: out_proj_blk(q, 0))
                add_fill(18 + qc, 4, lambda q=2 * qc: out_proj_blk(q, 1))
                add_fill(18 + qc, 5, lambda q=2 * qc + 1: out_proj_blk(q, 0))
                add_fill(18 + qc, 7, lambda q=2 * qc + 1: out_proj_blk(q, 1))

            # preamble v DMAs (phase-0 j1 projects v0/v1)
            v_load4(0)
            if KDEBUG:
                nc.sync.dma_start(VQD[:, :], vts[0].tensor[:, :, :].rearrange("p m d -> p (m d)"))
            v_load4(1)

            # ---- 24 attention phases + tail ----
            prev = None
            pending_tp = None
            exp_idx = 0
            for p in range(NPH):
                h, qc = p // 8, p % 8
                c, poff = HSL[h]
                last = p == NPH - 1
                etiles = []
                if prev is not None:
                    prev.pht = ps_ph.tile([P, 512], F32, tag="ph")
                for j in range(NJ):
                    for cb in fill.get((p, j), ()):
                        cb()
                    if j == 1 and pending_tp is not None:
                        normalize_b(pending_tp)
                        pending_tp = None
                    if prev is not None:
                        emit_chains_j(prev, j)
                    s = ps_s.tile([P, 4, QCH], F32, tag="s2")
                    for t in range(4):
                        kt = 4 * j + t
                        nc.tensor.matmul(
                            s[:, t, :],
                            kT_sb[poff:poff + DK, c, kt * P:(kt + 1) * P],
                            qT_sb[poff:poff + DK, c, qc * QCH:(qc + 1) * QCH],
                            start=True, stop=True,
                        )
                    e = epool.tile([P, 4, QCH], BF16, tag="e")
                    if j in (2, 5, 7):
                        with nc.allow_low_precision(reason="fast exp bf16 bits"):
                            nc.vector.tensor_scalar(
                                e.bitcast(I16), s, EXPA, EXPB, ALU.mult, ALU.add)
                    else:
                        nc.scalar.activation(e, s, FP.Exp, scale=0.125)
                    exp_idx += 1
                    etiles.append(e)
                if prev is not None:
                    normalize_a(prev)
                    pending_tp = prev
                prev = Ph(h, qc, etiles)

            # tail: last phase's chains + normalize + final out-proj; y
            # copies ride the idle ACT engine, Y DMAs spread across queues
            prev.pht = ps_ph.tile([P, 512], F32, tag="ph")
            for j in range(NJ):
                if j == 1 and pending_tp is not None:
                    normalize_b(pending_tp)
                    pending_tp = None
                emit_chains_j(prev, j)
                if j == 2:
                    out_proj_blk(12, 0)
                if j == 4:
                    out_proj_blk(12, 1)
                if j == 5:
                    out_proj_blk(13, 0)
                if j == 7:
                    out_proj_blk(13, 1)
            normalize_a(prev)
            normalize_b(prev)
            tail_eng = [nc.sync, nc.scalar, nc.gpsimd]
            for i, qt in enumerate(range(14, NQ // P)):
                y_sb = ypool.tile([P, D], F32, tag="y", name="y_sb")
                for n0, nsz in ((0, 512), (512, 256)):
                    ps = ps_s.tile([P, 512], F32, tag="s2")
                    nc.tensor.matmul(
                        ps[:, 0:nsz], oT_sb[:, 0, qt * P:(qt + 1) * P],
                        wo_sb[:, 0, n0:n0 + nsz], start=True, stop=False,
                    )
                    nc.tensor.matmul(
                        ps[:, 0:nsz], oT_sb[0:DK, 1, qt * P:(qt + 1) * P],
                        wo_sb[0:DK, 1, n0:n0 + nsz], start=False, stop=True,
                    )
                    nc.scalar.copy(y_sb[:, n0:n0 + nsz], ps[:, 0:nsz])
                tail_eng[i % 3].dma_start(Y[qt * P:(qt + 1) * P, :], y_sb)
            if KDEBUG:
                nc.sync.dma_start(KTD[:, :, :], kT_sb)
                nc.sync.dma_start(QTD[:, :, :], qT_sb)
                nc.sync.dma_start(VD[:, :, :], v_sb)
                nc.sync.dma_start(OTD[:, :, :], oT_sb)

    nc.compile()
    return nc


def kernel(**inputs):
    global _CACHED_NC, LAST_RESULTS
    bf = ml_dtypes.bfloat16
    f32 = np.float32

    Q = np.asarray(inputs["Q"], dtype=f32)
    K = np.asarray(inputs["K"], dtype=f32)
    V = np.asarray(inputs["V"], dtype=f32)
    Wq = np.asarray(inputs["Wq"], dtype=f32)
    bq = np.asarray(inputs["bq"], dtype=f32)
    Wk = np.asarray(inputs["Wk"], dtype=f32)
    bk = np.asarray(inputs["bk"], dtype=f32)
    Wv = np.asarray(inputs["Wv"], dtype=f32)
    bv = np.asarray(inputs["bv"], dtype=f32)
    Wo = np.asarray(inputs["Wo"], dtype=f32)
    bo = np.asarray(inputs["bo"], dtype=f32)

    KTh = np.ascontiguousarray(K[0].T).astype(bf)                 # [768, 4096]
    # per-tile layout [dim-in-block on partitions, position in tile on free]
    # so vt[:, kt*P:(kt+1)*P] is a valid stationary (contraction = dims)
    VTb = V[0].reshape(NKT, P, KD, P).transpose(0, 3, 2, 1).reshape(NKT, P, D)
    VTb4 = np.ascontiguousarray(
        VTb.reshape(NKT // 4, 4, P, D).transpose(0, 2, 1, 3).reshape(NKT // 4, P, 4 * D)
    ).astype(bf)
    QTs = [
        np.ascontiguousarray(Q[0, sq * NQ:(sq + 1) * NQ, :].T).astype(bf)
        for sq in range(2)
    ]
    IDN = np.eye(P, dtype=bf)

    def pad_bias(b):
        out = np.zeros((P, 2), dtype=f32)
        out[:, 0] = b[0:128]
        out[0:DK, 1] = b[128:192]
        return out

    def pack_w(wT):  # [768, 192] -> [128, 6*192] partition-major
        return np.ascontiguousarray(
            wT.reshape(KD, P, 192).transpose(1, 0, 2).reshape(P, KD * 192)
        ).astype(bf)

    per_hg = []
    for hg in range(4):
        sl = slice(hg * 192, (hg + 1) * 192)
        wo_pad = np.zeros((2 * P, D), dtype=f32)
        wo_pad[0:192] = Wo[:, sl].T
        bias = np.zeros((P, 4 + 192), dtype=f32)
        bias[:, 0:2] = pad_bias(bk[sl])
        bias[:, 2:4] = pad_bias(bq[sl])
        bias[:, 4:] = np.broadcast_to(bv[sl][None, :], (P, 192))
        per_hg.append(dict(
            WKB=np.ascontiguousarray(np.concatenate(
                [pack_w(Wk[sl, :].T),
                 bias.view(ml_dtypes.bfloat16).reshape(P, -1)], axis=1)),
            WQV=np.ascontiguousarray(
                np.concatenate([pack_w(Wq[sl, :].T), pack_w(Wv[sl, :].T)],
                               axis=1)),
            WoT=np.ascontiguousarray(
                wo_pad.reshape(2, P, D).transpose(1, 0, 2).reshape(P, 2 * D)
            ).astype(bf),
        ))

    in_maps = []
    for c in range(8):
        hg, sq = c // 2, c % 2
        in_maps.append(dict(
            KT=KTh, VTb4=VTb4, QT=QTs[sq], IDN=IDN, **per_hg[hg],
        ))

    if _CACHED_NC is None:
        _CACHED_NC = build_program()
    nc = _CACHED_NC

    LAST_RESULTS = run_bass_kernel_spmd(nc, in_maps, core_ids=list(range(8)))
    parts = [r["Y"] for r in LAST_RESULTS.results]

    out = np.empty((1, NPOS, D), dtype=f32)
    for sq in range(2):
        acc = parts[sq].astype(np.float64)
        for hg in range(1, 4):
            acc = acc + parts[2 * hg + sq]
        out[0, sq * NQ:(sq + 1) * NQ] = acc + bo[None, :]
    return out


# revision 6
# speedup vs baseline: 1.0074x; 1.0074x over previous
"""Multi-head attention (B=1, n=4096, d=768, H=12) on 8 Trainium2 NeuronCores.

Sharding: 4 head-groups (3 heads = 192 dims) x 2 query-halves (2048 q).
Core c = (hg = c // 2, sq = c % 2).

Cost-model-driven design (TimelineSim charges a matmul = moving-row count
regardless of stationary width, so output partitions should be 128 wide):

Per core:
  kT = Wk_hg @ K^T          [192, 4096]   2 partition-chunks (128+64)
  qT = Wq_hg @ Q_sq^T       [192, 2048]
  v  = V @ Wv_hg^T (+ ones) [4096, 3*65]  (65th col = 1.0 per head)
  Per phase (head h, 256-query chunk qc), 24 phases:
    S^T[kpos128, q256] = kT_h^T q_h       (dk=64 contraction, 8 PSUM tiles
                                           of 4 key-tiles each)
    e = exp(S^T/8)    ACT engine, bf16; 3 of 8 tiles per phase (j=2,5,7)
                      computed instead on DVE via a Schraudolph bf16-bits
                      fast exp (i16 round-convert of A*s+B, bitcast to bf16;
                      RMS rel err 1.8% on 37.5% of keys -> ~1.1% output
                      contribution, inside the 2e-2 gate)
    attnV (next phase, E-stationary): acc[q128, 65] += e[:,qb]^T [v_h|1]
      as 2 sequential 32-step PSUM chains (65-row matmuls, full 128 output
      partitions - half the PE cost of the v-stationary form)
    normalize: r = 1/acc[:,64] (DVE), o = acc[:,:64]*r -> bf16,
      PE-transpose (identity) -> oT[dims, q], DVE copy to oT_sb
  Y_part[2048, 768] = oT^T @ WoT_hg  (two dim-chunks accumulated in PSUM)

Host: Y[sq-half] = sum over 4 head-groups + bo.

Projections/out-proj are spread into the attention phases as PE filler so
the ACT exp stream and PE stay co-busy; PSUM: 3x2-bank tiles for scores/
proj/out-proj + 2x1-bank phase tiles (attnV accumulators + transpose area).
"""

import os as _os
import numpy as np
import ml_dtypes
KDEBUG = bool(_os.environ.get("KDEBUG"))

import concourse.bass as bass  # noqa: F401
import concourse.mybir as mybir
import concourse.tile as tile
from concourse import bacc
from concourse.bass_utils import run_bass_kernel_spmd

P = 128
D = 768
NPOS = 4096
NQ = 2048          # queries per core
KD = D // P        # 6 contraction tiles for projections
NKT = NPOS // P    # 32 key-position tiles
DK = 64
VW = 65            # v columns per head incl. ones column
QCH = 256          # query chunk per attention phase
NJ = 8             # j-steps per phase (4 key-tiles each)
HC = 3             # heads per core
NPH = HC * NPOS // QCH // 2  # 24 phases (qc covers NQ=2048 -> 8 chunks/head)
BF16 = mybir.dt.bfloat16
F32 = mybir.dt.float32
I16 = mybir.dt.int16
FP = mybir.ActivationFunctionType
ALU = mybir.AluOpType

# head h -> (dim-chunk, partition offset)
HSL = [(0, 0), (0, 64), (1, 0)]
CW = [128, 64]     # chunk widths

# Schraudolph fast-exp constants (bf16 bits = round(s*EXPA + EXPB)), folding
# the 1/sqrt(dk)=1/8 score scale; calibrated for round-to-nearest convert.
EXPA = float(16.0 * np.log2(np.e))
EXPB = float(127.0 * 128.0 - 7.43)
FAST_EVERY = 4     # every FAST_EVERY-th exp tile goes to DVE fast-exp

_CACHED_NC = None
LAST_RESULTS = None  # BassKernelResults from the most recent run (for test.py)


def build_program():
    nc = bacc.Bacc("TRN2", target_bir_lowering=False, debug=False)

    KT = nc.dram_tensor("KT", [D, NPOS], BF16, kind="ExternalInput")
    VTb4 = nc.dram_tensor("VTb4", [NKT // 4, P, 4 * D], BF16, kind="ExternalInput")
    QT = nc.dram_tensor("QT", [D, NQ], BF16, kind="ExternalInput")
    # weights pre-packed on host to [128, k*m] (partition-major) for
    # contiguous DMA runs
    NB2 = 2 * (4 + HC * DK)   # bias f32 block carried as bf16 columns
    WKB = nc.dram_tensor("WKB", [P, KD * HC * DK + NB2], BF16, kind="ExternalInput")
    WQV = nc.dram_tensor("WQV", [P, 2 * KD * HC * DK], BF16, kind="ExternalInput")
    WoT = nc.dram_tensor("WoT", [P, 2 * D], BF16, kind="ExternalInput")
    IDN = nc.dram_tensor("IDN", [P, P], BF16, kind="ExternalInput")
    Y = nc.dram_tensor("Y", [NQ, D], BF16, kind="ExternalOutput")
    if KDEBUG:
        KTD = nc.dram_tensor("KTD", [P, 2, NPOS], BF16, kind="ExternalOutput")
        QTD = nc.dram_tensor("QTD", [P, 2, NQ], BF16, kind="ExternalOutput")
        VD = nc.dram_tensor("VD", [P, NKT, HC * VW], BF16, kind="ExternalOutput")
        OTD = nc.dram_tensor("OTD", [P, 2, NQ], BF16, kind="ExternalOutput")
        VQD = nc.dram_tensor("VQD", [P, 4 * D], BF16, kind="ExternalOutput")

    with tile.TileContext(nc) as tc:
        with (
            tc.tile_pool(name="const", bufs=1) as const,
            tc.tile_pool(name="persist", bufs=1) as persist,
            tc.tile_pool(name="vin", bufs=4) as vin,
            tc.tile_pool(name="epool", bufs=17) as epool,
            tc.tile_pool(name="small", bufs=4) as small,
            tc.tile_pool(name="ypool", bufs=2) as ypool,
            tc.tile_pool(name="ps_s", bufs=3, space="PSUM") as ps_s,
            tc.tile_pool(name="ps_ph", bufs=2, space="PSUM") as ps_ph,
        ):
            # ---- SBUF tiles (DMAs emitted in need-order below) ----
            wqv_sb = const.tile([P, 2, KD, HC * DK], BF16)
            wq_sb = wqv_sb[:, 0]
            wv_sb = wqv_sb[:, 1]
            wkb_sb = const.tile([P, KD * HC * DK + NB2], BF16)
            wk_sb = wkb_sb[:, 0:KD * HC * DK].rearrange(
                "p (k m) -> p k m", k=KD)
            bias_sb = wkb_sb[:, KD * HC * DK:].bitcast(F32)
            bk_sb = bias_sb[:, 0:2]
            bq_sb = bias_sb[:, 2:4]
            bvr_sb = bias_sb[:, 4:]
            wo_sb = const.tile([P, 2, D], BF16)
            idn = const.tile([P, P], BF16)

            KT_res = persist.tile([P, KD, NPOS], BF16)
            QT_res = persist.tile([P, KD, NQ], BF16)
            kT_sb = persist.tile([P, 2, NPOS], BF16)
            qT_sb = persist.tile([P, 2, NQ], BF16)
            v_sb = persist.tile([P, NKT, HC * VW], BF16)
            oT_sb = persist.tile([P, 2, NQ], BF16)

            v_heads = v_sb.rearrange("p m (h x) -> p m h x", x=VW)
            nc.vector.memset(v_heads[:, :, :, DK], 1.0)

            KT_r = KT.rearrange("(k p) (t n) -> p k t n", p=P, n=512)
            QT_r = QT.rearrange("(k p) (t n) -> p k t n", p=P, n=512)

            # SP queue: wk, biases, KT pieces, wq/wv; Pool-issued (SWDGE)
            # queue: idn, QT pieces, VTb quads, wo. Host pre-packs weights
            # partition-major so every DMA has long contiguous runs.
            nc.sync.dma_start(wkb_sb, WKB[:, :])
            nc.sync.dma_start(KT_res[:, :, 0:512], KT_r[:, :, 0])
            nc.sync.dma_start(
                wqv_sb.rearrange("p a k m -> p (a k m)"), WQV[:, :])
            nc.gpsimd.dma_start(QT_res[:, :, 0:512], QT_r[:, :, 0])
            nc.gpsimd.dma_start(idn, IDN[:, :])

            def load_kt(nt):
                nc.sync.dma_start(KT_res[:, :, nt * 512:(nt + 1) * 512], KT_r[:, :, nt])

            def load_kt2(pc):  # 1024-wide KT piece (nt 2pc+1, 2pc+2)
                nc.sync.dma_start(
                    KT_res[:, :, (2 * pc + 1) * 512:(2 * pc + 3) * 512],
                    KT_r.rearrange("p k t n -> p k (t n)")[
                        :, :, (2 * pc + 1) * 512:(2 * pc + 3) * 512],
                )

            def load_qt(nt):
                nc.gpsimd.dma_start(QT_res[:, :, nt * 512:(nt + 1) * 512], QT_r[:, :, nt])

            vts = {}

            def v_load4(qd):  # quad of VTb tiles on the Pool/SWDGE queue
                vtq = vin.tile([P, 4, D], BF16, tag="vin")
                nc.gpsimd.dma_start(vtq.rearrange("p m d -> p (m d)"), VTb4[qd])
                for i in range(4):
                    vts[4 * qd + i] = vtq[:, i, :]

            # trigger the exp table load while input DMAs stream
            warm_in = const.tile([1, DK], F32)
            nc.vector.memset(warm_in, 0.0)
            warm_sb = const.tile([1, DK], F32)
            nc.scalar.activation(warm_sb, warm_in, FP.Exp)

            def proj_chunk(c, w_sb, b_sb, dst, nt):
                """Project 512 positions of dim-chunk c of k or q (+bias)."""
                src = KT_res if dst is kT_sb else QT_res
                cw = CW[c]
                ps = ps_s.tile([P, 512], F32, tag="s2")
                for kt in range(KD):
                    nc.tensor.matmul(
                        ps[0:cw, :], w_sb[:, kt, c * P:c * P + cw],
                        src[:, kt, nt * 512:(nt + 1) * 512],
                        start=(kt == 0), stop=(kt == KD - 1),
                    )
                nc.vector.tensor_scalar_add(
                    dst[0:cw, c, nt * 512:(nt + 1) * 512], ps[0:cw, :],
                    b_sb[0:cw, c:c + 1],
                )

            def v_proj(mt):
                vt = vts.pop(mt)  # [P, D] slice of a quad tile
                ps = ps_s.tile([P, 512], F32, tag="s2")
                for kt in range(KD):
                    nc.tensor.matmul(
                        ps[:, 0:HC * DK], vt[:, kt * P:(kt + 1) * P],
                        wv_sb[:, kt, :],
                        start=(kt == 0), stop=(kt == KD - 1),
                    )
                nc.vector.tensor_tensor(
                    v_heads[:, mt, :, 0:DK],
                    ps[:, 0:HC * DK].rearrange("p (h x) -> p h x", x=DK),
                    bvr_sb.rearrange("p (h x) -> p h x", x=DK),
                    ALU.add,
                )

            ypend = {}

            def out_proj_blk(qt, blk):
                if blk == 0:
                    ypend[qt] = ypool.tile([P, D], BF16, tag="y", name="y_sb")
                y_sb = ypend[qt]
                n0, nsz = ((0, 512), (512, 256))[blk]
                ps = ps_s.tile([P, 512], F32, tag="s2")
                nc.tensor.matmul(
                    ps[:, 0:nsz], oT_sb[:, 0, qt * P:(qt + 1) * P],
                    wo_sb[:, 0, n0:n0 + nsz], start=True, stop=False,
                )
                nc.tensor.matmul(
                    ps[:, 0:nsz], oT_sb[0:DK, 1, qt * P:(qt + 1) * P],
                    wo_sb[0:DK, 1, n0:n0 + nsz], start=False, stop=True,
                )
                nc.vector.tensor_copy(y_sb[:, n0:n0 + nsz], ps[:, 0:nsz])
                if blk == 1:
                    nc.sync.dma_start(Y[qt * P:(qt + 1) * P, :], ypend.pop(qt))

            def out_proj(qt):
                out_proj_blk(qt, 0)
                out_proj_blk(qt, 1)

            # ---- phase machinery ----
            class Ph:
                def __init__(self, h, qc, etiles):
                    self.h, self.qc, self.e = h, qc, etiles
                    self.pht = None

            def emit_chains_j(prev, j):
                """8 attnV matmuls of the previous phase at j-step j."""
                h, qc = prev.h, prev.qc
                qb = j // 4
                for m in range(8):
                    kt = 8 * (j % 4) + m
                    et = prev.e[kt // 4]
                    nc.tensor.matmul(
                        prev.pht[:, qb * VW:qb * VW + VW],
                        et[:, kt % 4, qb * P:(qb + 1) * P],
                        v_heads[:, kt, h, :],
                        start=(kt == 0), stop=(kt == NKT - 1),
                    )

            def normalize_a(prev):
                """DVE part: reciprocal of the ones-column, scale to bf16."""
                prev.on = []
                for qb in range(2):
                    r = small.tile([P, 1], F32, tag="r")
                    with nc.allow_low_precision(reason="softmax reciprocal"):
                        nc.vector.reciprocal(
                            r, prev.pht[:, qb * VW + DK:qb * VW + DK + 1])
                    o_n = small.tile([P, DK], BF16, tag="on")
                    nc.vector.tensor_scalar(
                        o_n, prev.pht[:, qb * VW:qb * VW + DK], r, None, ALU.mult)
                    prev.on.append(o_n)

            def normalize_b(prev):
                """PE transpose (deferred so PE never waits on the DVE part)
                + DVE copy into oT_sb."""
                h, qc = prev.h, prev.qc
                c, poff = HSL[h]
                for qb in range(2):
                    tp = prev.pht[poff:poff + DK,
                                  384 + qb * DK:384 + (qb + 1) * DK].bitcast(BF16)
                    nc.tensor.transpose(tp, prev.on[qb], idn)
                    nc.vector.tensor_copy(
                        oT_sb[poff:poff + DK, c,
                              qc * QCH + qb * P:qc * QCH + (qb + 1) * P],
                        tp,
                    )

            # ---- filler schedule: (phase, j) -> list of callbacks ----
            fill = {}

            def add_fill(p, j, cb):
                fill.setdefault((p, j), []).append(cb)

            def kchunk(c, nt):
                return lambda: proj_chunk(c, wk_sb, bk_sb, kT_sb, nt)

            def qchunk(c, nt):
                return lambda: proj_chunk(c, wq_sb, bq_sb, qT_sb, nt)

            # phase 0: kT chunk0 JIT (KT arriving in 1024-wide pieces) plus
            # the first 14 v projections; VTb quads staged on the Pool queue.
            for pc in range(3):
                add_fill(0, 2 * pc, lambda n=pc: load_kt2(n))
            add_fill(0, 0, lambda: load_kt(7))
            for j in range(8):
                add_fill(0, j, kchunk(0, j))
            add_fill(0, 0, qchunk(0, 0))
            add_fill(0, 1, lambda: v_load4(2))
            add_fill(0, 3, lambda: v_load4(3))
            add_fill(0, 5, lambda: v_load4(4))
            add_fill(0, 7, lambda: v_load4(5))
            for j in range(1, 8):
                add_fill(0, j, lambda m=2 * j - 2: v_proj(m))
                add_fill(0, j, lambda m=2 * j - 1: v_proj(m))
            # phase 1: v14..31 (5,5,5,3 over j0..3 - chain j needs v<8(j+1))
            add_fill(1, 0, lambda: load_qt(1))
            add_fill(1, 0, lambda: v_load4(6))
            add_fill(1, 1, lambda: v_load4(7))
            vnext = 14
            for j in range(4):
                for _ in range(5 if j < 3 else 3):
                    add_fill(1, j, lambda m=vnext: v_proj(m))
                    vnext += 1
            add_fill(1, 5, qchunk(0, 1))      # needed by phase 2
            add_fill(2, 0, lambda: load_qt(2))
            add_fill(3, 5, qchunk(0, 2))
            add_fill(3, 0, lambda: load_qt(3))
            add_fill(5, 5, qchunk(0, 3))
            for i in range(8):                # kT chunk1: needed by phase 16
                add_fill(2 + i, 5, kchunk(1, i))
            for i in range(4):                # qT chunk1: needed by phase 16+2i
                add_fill(10 + i, 5, qchunk(1, i))
            add_fill(15, 5, lambda: nc.gpsimd.dma_start(
                wo_sb.rearrange("p k m -> p (k m)"), WoT[:, :]))
            for qc in range(6):               # out-proj qt pairs, phases 18..23
                add_fill(18 + qc, 2, lambda q=2 * qc: out_proj_blk(q, 0))
                add_fill(18 + qc, 4, lambda q=2 * qc: out_proj_blk(q, 1))
                add_fill(18 + qc, 5, lambda q=2 * qc + 1: out_proj_blk(q, 0))
                add_fill(18 + qc, 7, lambda q=2 * qc + 1: out_proj_blk(q, 1))

            # preamble v DMAs (phase-0 j1 projects v0/v1)
            v_load4(0)
            if KDEBUG:
                nc.sync.dma_start(VQD[:, :], vts[0].tensor[:, :, :].rearrange("p m d -> p (m d)"))
            v_load4(1)

            # ---- 24 attention phases + tail ----
            prev = None
            pending_tp = None
            exp_idx = 0
            for p in range(NPH):
                h, qc = p // 8, p % 8
                c, poff = HSL[h]
                last = p == NPH - 1
                etiles = []
                if prev is not None:
                    prev.pht = ps_ph.tile([P, 512], F32, tag="ph")
                for j in range(NJ):
                    for cb in fill.get((p, j), ()):
                        cb()
                    if j == 1 and pending_tp is not None:
                        normalize_b(pending_tp)
                        pending_tp = None
                    if prev is not None:
                        emit_chains_j(prev, j)
                    s = ps_s.tile([P, 4, QCH], F32, tag="s2")
                    for t in range(4):
                        kt = 4 * j + t
                        nc.tensor.matmul(
                            s[:, t, :],
                            kT_sb[poff:poff + DK, c, kt * P:(kt + 1) * P],
                            qT_sb[poff:poff + DK, c, qc * QCH:(qc + 1) * QCH],
                            start=True, stop=True,
                        )
                    e = epool.tile([P, 4, QCH], BF16, tag="e")
                    if j in (2, 5, 7):
                        with nc.allow_low_precision(reason="fast exp bf16 bits"):
                            nc.vector.tensor_scalar(
                                e.bitcast(I16), s, EXPA, EXPB, ALU.mult, ALU.add)
                    else:
                        nc.scalar.activation(e, s, FP.Exp, scale=0.125)
                    exp_idx += 1
                    etiles.append(e)
                if prev is not None:
                    normalize_a(prev)
                    pending_tp = prev
                prev = Ph(h, qc, etiles)

            # tail: last phase's chains + normalize + final out-proj; y
            # copies ride the idle ACT engine, Y DMAs spread across queues
            prev.pht = ps_ph.tile([P, 512], F32, tag="ph")
            for j in range(NJ):
                if j == 1 and pending_tp is not None:
                    normalize_b(pending_tp)
                    pending_tp = None
                emit_chains_j(prev, j)
                if j == 2:
                    out_proj_blk(12, 0)
                if j == 4:
                    out_proj_blk(12, 1)
                if j == 5:
                    out_proj_blk(13, 0)
                if j == 7:
                    out_proj_blk(13, 1)
            normalize_a(prev)
            normalize_b(prev)
            tail_eng = [nc.sync, nc.scalar, nc.gpsimd]
            for i, qt in enumerate(range(14, NQ // P)):
                y_sb = ypool.tile([P, D], BF16, tag="y", name="y_sb")
                for n0, nsz in ((0, 512), (512, 256)):
                    ps = ps_s.tile([P, 512], F32, tag="s2")
                    nc.tensor.matmul(
                        ps[:, 0:nsz], oT_sb[:, 0, qt * P:(qt + 1) * P],
                        wo_sb[:, 0, n0:n0 + nsz], start=True, stop=False,
                    )
                    nc.tensor.matmul(
                        ps[:, 0:nsz], oT_sb[0:DK, 1, qt * P:(qt + 1) * P],
                        wo_sb[0:DK, 1, n0:n0 + nsz], start=False, stop=True,
                    )
                    nc.scalar.copy(y_sb[:, n0:n0 + nsz], ps[:, 0:nsz])
                tail_eng[i % 3].dma_start(Y[qt * P:(qt + 1) * P, :], y_sb)
            if KDEBUG:
                nc.sync.dma_start(KTD[:, :, :], kT_sb)
                nc.sync.dma_start(QTD[:, :, :], qT_sb)
                nc.sync.dma_start(VD[:, :, :], v_sb)
                nc.sync.dma_start(OTD[:, :, :], oT_sb)

    nc.compile()
    return nc


def kernel(**inputs):
    global _CACHED_NC, LAST_RESULTS
    bf = ml_dtypes.bfloat16
    f32 = np.float32

    Q = np.asarray(inputs["Q"], dtype=f32)
    K = np.asarray(inputs["K"], dtype=f32)
    V = np.asarray(inputs["V"], dtype=f32)
    Wq = np.asarray(inputs["Wq"], dtype=f32)
    bq = np.asarray(inputs["bq"], dtype=f32)
    Wk = np.asarray(inputs["Wk"], dtype=f32)
    bk = np.asarray(inputs["bk"], dtype=f32)
    Wv = np.asarray(inputs["Wv"], dtype=f32)
    bv = np.asarray(inputs["bv"], dtype=f32)
    Wo = np.asarray(inputs["Wo"], dtype=f32)
    bo = np.asarray(inputs["bo"], dtype=f32)

    KTh = np.ascontiguousarray(K[0].T).astype(bf)                 # [768, 4096]
    # per-tile layout [dim-in-block on partitions, position in tile on free]
    # so vt[:, kt*P:(kt+1)*P] is a valid stationary (contraction = dims)
    VTb = V[0].reshape(NKT, P, KD, P).transpose(0, 3, 2, 1).reshape(NKT, P, D)
    VTb4 = np.ascontiguousarray(
        VTb.reshape(NKT // 4, 4, P, D).transpose(0, 2, 1, 3).reshape(NKT // 4, P, 4 * D)
    ).astype(bf)
    QTs = [
        np.ascontiguousarray(Q[0, sq * NQ:(sq + 1) * NQ, :].T).astype(bf)
        for sq in range(2)
    ]
    IDN = np.eye(P, dtype=bf)

    def pad_bias(b):
        out = np.zeros((P, 2), dtype=f32)
        out[:, 0] = b[0:128]
        out[0:DK, 1] = b[128:192]
        return out

    def pack_w(wT):  # [768, 192] -> [128, 6*192] partition-major
        return np.ascontiguousarray(
            wT.reshape(KD, P, 192).transpose(1, 0, 2).reshape(P, KD * 192)
        ).astype(bf)

    per_hg = []
    for hg in range(4):
        sl = slice(hg * 192, (hg + 1) * 192)
        wo_pad = np.zeros((2 * P, D), dtype=f32)
        wo_pad[0:192] = Wo[:, sl].T
        bias = np.zeros((P, 4 + 192), dtype=f32)
        bias[:, 0:2] = pad_bias(bk[sl])
        bias[:, 2:4] = pad_bias(bq[sl])
        bias[:, 4:] = np.broadcast_to(bv[sl][None, :], (P, 192))
        per_hg.append(dict(
            WKB=np.ascontiguousarray(np.concatenate(
                [pack_w(Wk[sl, :].T),
                 bias.view(ml_dtypes.bfloat16).reshape(P, -1)], axis=1)),
            WQV=np.ascontiguousarray(
                np.concatenate([pack_w(Wq[sl, :].T), pack_w(Wv[sl, :].T)],
                               axis=1)),
            WoT=np.ascontiguousarray(
                wo_pad.reshape(2, P, D).transpose(1, 0, 2).reshape(P, 2 * D)
            ).astype(bf),
        ))

    in_maps = []
    for c in range(8):
        hg, sq = c // 2, c % 2
        in_maps.append(dict(
            KT=KTh, VTb4=VTb4, QT=QTs[sq], IDN=IDN, **per_hg[hg],
        ))

    if _CACHED_NC is None:
        _CACHED_NC = build_program()
    nc = _CACHED_NC

    LAST_RESULTS = run_bass_kernel_spmd(nc, in_maps, core_ids=list(range(8)))
    parts = [r["Y"] for r in LAST_RESULTS.results]

    out = np.empty((1, NPOS, D), dtype=f32)
    for sq in range(2):
        acc = parts[sq].astype(np.float64)
        for hg in range(1, 4):
            acc = acc + parts[2 * hg + sq].astype(np.float64)
        out[0, sq * NQ:(sq + 1) * NQ] = acc + bo[None, :]
    return out
